# revision 5
# baseline (speedup 1.0000x reference)
"""GPTQ int4 quantized linear (CaiQuantLinear) on 8 Trainium2 NeuronCores.

y = x @ dequant(qweight, scales, qzeros) + bias
  x: [8192, 4096] f32, qweight: [256, 4096] int64 (16x 4-bit packed along
  infeatures), scales: [32, 4096] f32, qzeros: [32, 256] int64 (packed along
  outfeatures), g_idx = arange(4096)//128, bias: [4096] f32 -> y: [8192, 4096] f32

Sharding: 4 token-shards x 2 outfeature-shards = 8 cores. Core c handles
tokens [2048*(c//2), +2048) and outfeatures [2048*(c%2), +2048).

Device kernel (per core): the weight stream ships one byte per nibble with
the 4-bit code in the HIGH bits (host bit-shuffle only) fused per-row with
the bf16 scale/zero rows ([q u8 | s/16 | 16z] per contraction tile), so
dequant is two tensor_tensor ops: (q - 16z) * (s/16). The stream is
5.25MB per outfeature block (vs 50MB when shipping int16 codes).

The DMA fabric sustains ~400GB/s aggregate per core, so the load phase
(x shard 16.8MB + first weights) is throughput-bound. To cover it, the
first TWO outfeature blocks are computed as one interleaved phase: 112us
of PE work against ~70us of fill, with the ob0 stream two token-groups
ahead of ob1. Matmuls accumulate 32 k-tiles of [128,128]x[128,256] bf16
into PSUM; evacuation adds the bias.
"""

import sys

if "/opt/trn_rl_repo" not in sys.path:
    sys.path.insert(0, "/opt/trn_rl_repo")

import numpy as np
import ml_dtypes

import concourse.bass as bass  # noqa: F401  (registers mybir types)
import concourse.mybir as mybir
import concourse.tile as tile
from concourse import bacc
from concourse.bass_utils import run_bass_kernel_spmd

BF16 = mybir.dt.bfloat16
F32 = mybir.dt.float32
U8 = mybir.dt.uint8

N_CORES = 8
NT, NO = 4, 2          # token shards x outfeature shards
TOK, IN_F, OUT_F = 8192, 4096, 4096
T = TOK // NT          # 2048 tokens per core
OS = OUT_F // NO       # 2048 outfeatures per core
P = 128
NB = IN_F // P         # 32 contraction super-tiles
OB = 256               # outfeature block (psum free dim)
NOB = OS // OB         # 8
NTB = T // P           # 16 token blocks

CB = 4                 # super-tiles per weight-stream chunk
NCH = NB // CB         # 8 chunks per outfeature block
BLK = OB + 4 * OB      # 1280 bytes per b: [q u8 | s bf16 | z bf16]

_CACHE = {}


def _build_program():
    nc = bacc.Bacc("TRN2", target_bir_lowering=False, debug=False,
                   num_devices=N_CORES)
    xt_ap = nc.dram_tensor("xt", [NTB, P, NB, P], BF16, kind="ExternalInput").ap()
    pk_ap = nc.dram_tensor("pk", [NOB, NCH, P, CB * BLK], U8,
                           kind="ExternalInput").ap()
    br_ap = nc.dram_tensor("br", [OS], BF16, kind="ExternalInput").ap()
    y_ap = nc.dram_tensor("y", [NTB, NOB, P, OB], F32, kind="ExternalOutput").ap()

    with tile.TileContext(nc) as tc:
        with tc.tile_pool(name="resident", bufs=1) as rpool, \
             tc.tile_pool(name="wset", bufs=3) as wpool, \
             tc.tile_pool(name="qstream", bufs=4) as qpool, \
             tc.tile_pool(name="ostream", bufs=4) as opool, \
             tc.tile_pool(name="psum", bufs=6, space="PSUM") as ppool, \
             tc.tile_pool(name="jpsum", bufs=1, space="PSUM") as jpool:
            br_sb = rpool.tile([P, OS], BF16)
            nc.sync.dma_start(br_sb[:], br_ap.partition_broadcast(P))
            # zeros rhs for PE-warmup matmuls during the load phase
            wz = rpool.tile([P, OB], BF16)
            nc.gpsimd.memset(wz[:], 0.0)
            jp = jpool.tile([P, OB], F32)
            xt_sb = rpool.tile([P, NTB, NB, P], BF16)

            # warm the PE while the first weights/x stream in
            for _ in range(2):
                nc.tensor.matmul(jp[:], wz[:, :P], wz[:], start=True, stop=True)

            def dequant(wset, pk_sb, ch):
                for l in range(CB):
                    b = ch * CB + l
                    base = l * BLK
                    qt = pk_sb[:, base:base + OB]
                    st = pk_sb[:, base + OB:base + 3 * OB].bitcast(BF16)
                    zt = pk_sb[:, base + 3 * OB:base + 5 * OB].bitcast(BF16)
                    tmp = qpool.tile([P, OB], BF16, tag="tmp")
                    nc.vector.tensor_tensor(
                        tmp[:], qt, zt, mybir.AluOpType.subtract)
                    nc.vector.tensor_tensor(
                        wset[:, b, :], tmp[:], st, mybir.AluOpType.mult)

            # --- head: stream ob0+ob1 weights interleaved with x
            wsets = [wpool.tile([P, NB, OB], BF16, tag="wset", name=f"w{o}")
                     for o in range(2)]
            for ch in range(NCH):
                for o in range(2):
                    pk_sb = qpool.tile([P, CB * BLK], U8, tag="pk")
                    eng = nc.sync if o == 0 else nc.scalar
                    eng.dma_start(pk_sb[:], pk_ap[o, ch])
                    if ch == 0:
                        # junk matmul on arrived bytes keeps the PE p-state
                        # ramping before the first dequanted weights exist
                        nc.tensor.matmul(
                            jp[:], pk_sb[:, :2 * P].bitcast(BF16), wz[:],
                            start=True, stop=True)
                    dequant(wsets[o], pk_sb, ch)
                eng = nc.sync if ch % 2 else nc.scalar
                eng.dma_start(xt_sb[:, 2 * ch], xt_ap[2 * ch])
                eng2 = nc.scalar if ch % 2 else nc.sync
                eng2.dma_start(xt_sb[:, 2 * ch + 1], xt_ap[2 * ch + 1])

            def produce_wset(ob):
                wset = wpool.tile([P, NB, OB], BF16, tag="wset")
                for ch in range(NCH):
                    pk_sb = qpool.tile([P, CB * BLK], U8, tag="pk")
                    eng = nc.sync if ch % 2 else nc.scalar
                    eng.dma_start(pk_sb[:], pk_ap[ob, ch])
                    dequant(wset, pk_sb, ch)
                return wset

            def evac(pslice, tb, ob):
                ot = opool.tile([P, OB], F32, tag="ot")
                nc.vector.tensor_tensor(
                    ot[:], pslice, br_sb[:, ob * OB:(ob + 1) * OB],
                    mybir.AluOpType.add)
                nc.gpsimd.dma_start(y_ap[tb, ob], ot[:])

            def group(tb, ob, wset):
                ps = ppool.tile([P, OB], F32, tag="ps")
                for b in range(NB):
                    nc.tensor.matmul(
                        ps[:], xt_sb[:, tb, b, :], wset[:, b, :],
                        start=(b == 0), stop=(b == NB - 1))
                evac(ps[:], tb, ob)

            # paired head phase: ob0 leads ob1 by two token-groups so the
            # PE always has work while the x shard streams in
            for tb in range(NTB):
                group(tb, 0, wsets[0])
                if tb >= 2:
                    group(tb - 2, 1, wsets[1])
            for tb in range(NTB - 2, NTB):
                group(tb, 1, wsets[1])

            for ob in range(2, NOB):
                wset = produce_wset(ob)
                for tb in range(NTB):
                    group(tb, ob, wset)

    nc.compile()
    return nc


def _host_prep(x, qweight, scales, qzeros, bias):
    """Per-core input maps: layout prep only (transpose / nibble byte-split /
    row replication); dequantization happens on-chip."""
    bf16 = ml_dtypes.bfloat16
    x = np.asarray(x, dtype=np.float32)
    qw = np.asarray(qweight).astype(np.int64, copy=False)
    sc = np.asarray(scales, dtype=np.float32)
    qz = np.asarray(qzeros).astype(np.int64, copy=False)
    bi = np.asarray(bias, dtype=np.float32)

    # zeros: unpack along outfeatures, +1 (pack() stored z-1)
    shifts = (np.arange(16, dtype=np.uint64) * np.uint64(4))
    zz = ((qz.astype(np.uint64)[:, :, None] >> shifts[None, None, :])
          & np.uint64(15)).reshape(qz.shape[0], -1).astype(np.float32) + 1.0

    # per-token-shard xT: [NTB, P(k-part), NB, P(t)]
    xt_list = []
    for tc in range(NT):
        xs = x[tc * T:(tc + 1) * T]                      # [T, IN_F]
        xt = np.ascontiguousarray(xs.T).astype(bf16)     # [IN_F, T]
        xt_list.append(np.ascontiguousarray(
            xt.reshape(NB, P, NTB, P).transpose(2, 1, 0, 3)))

    # per-outfeature-shard fused weight stream [NOB, NCH, P, CB*BLK]:
    # per b: [q u8 (code<<4) | s/16 bf16 | 16z bf16]
    pk_list, br_list = [], []
    for oc in range(NO):
        o0 = oc * OS
        qsl = np.ascontiguousarray(qw[:, o0:o0 + OS])    # [256, OS] int64
        qbytes = qsl.view(np.uint8).reshape(IN_F // 16, OS, 8)
        qb2 = np.ascontiguousarray(qbytes.transpose(0, 2, 1)).reshape(IN_F // 2, OS)
        nib = np.empty((IN_F, OS), np.uint8)             # row k: code(k, o) << 4
        nib[0::2] = (qb2 & np.uint8(15)) << np.uint8(4)
        nib[1::2] = qb2 & np.uint8(0xF0)
        q_t = nib.reshape(NCH, CB, P, NOB, OB).transpose(3, 0, 2, 1, 4)

        s16 = (sc[:, o0:o0 + OS] / 16.0).astype(bf16).reshape(NB, NOB, OB)
        z16 = (zz[:, o0:o0 + OS] * 16.0).astype(bf16).reshape(NB, NOB, OB)
        st = s16.reshape(NCH, CB, NOB, OB).transpose(2, 0, 1, 3)
        zt = z16.reshape(NCH, CB, NOB, OB).transpose(2, 0, 1, 3)
        sb = np.broadcast_to(st[:, :, None], (NOB, NCH, P, CB, OB))
        zb = np.broadcast_to(zt[:, :, None], (NOB, NCH, P, CB, OB))
        blk = np.concatenate(
            [np.ascontiguousarray(q_t),
             np.ascontiguousarray(sb).view(np.uint8),
             np.ascontiguousarray(zb).view(np.uint8)], axis=-1)
        pk_list.append(np.ascontiguousarray(
            blk.reshape(NOB, NCH, P, CB * BLK)))
        br_list.append(np.ascontiguousarray(bi[o0:o0 + OS].astype(bf16)))

    in_maps = []
    for c in range(N_CORES):
        tc, oc = c // NO, c % NO
        in_maps.append({
            "xt": xt_list[tc],
            "pk": pk_list[oc],
            "br": br_list[oc],
        })
    return in_maps


def get_program():
    if "nc" not in _CACHE:
        _CACHE["nc"] = _build_program()
    return _CACHE["nc"]


def kernel(x, qweight, scales, qzeros, g_idx, bias):
    nc = get_program()
    in_maps = _host_prep(x, qweight, scales, qzeros, bias)
    res = run_bass_kernel_spmd(nc, in_maps, core_ids=list(range(N_CORES)))
    y = np.empty((TOK, OUT_F), dtype=np.float32)
    for c in range(N_CORES):
        tc, oc = c // NO, c % NO
        yt = res.results[c]["y"]                         # [NTB, NOB, P, OB]
        y[tc * T:(tc + 1) * T, oc * OS:(oc + 1) * OS] = (
            yt.transpose(0, 2, 1, 3).reshape(T, OS))
    return y
